# revision 32
# baseline (speedup 1.0000x reference)
"""Cross-attention (B=4, L=2048, D=1024, H=16) on 8 TRN2 NeuronCores.

Sharding: core c handles batch b = c//2 and head-group g = c%2 (8 heads,
512 projection features). Each core computes its heads' Q/K/V projections,
attention, and a partial output projection (contraction over its 512
features). Host sums the two partials per batch and adds the output bias.

Per-core layouts (host pre-arranged, matmul operands cast to bf16):
  xq/xk/xv [D=1024, L=2048]  activations transposed (contraction dim on
                             partitions for the projection matmuls), bf16
  wq/wk/wv [D=1024, F=512]   W[F,:].T  (d-major), bf16
  wo       [F=512, D=1024]   Wo[:,F].T (feat-major), bf16
  bqk      [2, 4, 128]       q/k biases reshaped for partition-dim loads
  bv       [512]             v bias (free-dim broadcast DMA)
Output: out [L=2048, D=1024] fp32 partial (x_g @ Wo[:,F].T), no bias.

On-device dataflow per core (all matmuls bf16 with fp32 psum accumulate):
  QT/KT [feat(4x128p), L] bf16 = (w-tile).T @ x-tile + bias
  V     [L(16x128p), 8*(V_h 64 | ones 64)] bf16 (ones interleaved so one
        M=128 matmul yields both attention output and softmax denominator)
  per (head, l_q chunk) unit, groups of 2 l_k tiles:
    E.T [l_k 128, 2*512] psum = KT_h_tile.T @ QT_h   (K=64)
    ACT exp(0.125 * E.T) reads both psum banks -> P.T bf16 in SBUF
        (no max subtraction: |E/8| < ~7 for these distributions)
    X'' [128, 512] psum += [V_h|ones].T @ P.T  (partitions 0:64 = X.T,
        64:128 = replicated denominator)
    DVE: rec = reciprocal(X''[64:128]); XT = X''[0:64] * rec
  The next unit's E groups are emitted interleaved with the current
  unit's X'' matmuls so ACT (the softmax bottleneck) never idles.
  out [l 128, j 512] psum = XT-tile.T @ wo, DVE copy, DMA -> DRAM
"""

from contextlib import ExitStack

import numpy as np
import ml_dtypes

import concourse.bass as bass
import concourse.tile as tile
import concourse.mybir as mybir
from concourse.bass_utils import run_bass_kernel_spmd

F32 = mybir.dt.float32
F32R = mybir.dt.float32r
BF16 = mybir.dt.bfloat16


class _TileContext(tile.TileContext):
    """TileContext whose kernel-tail drain splits its semaphore waits.

    The stock ``_drain_and_barrier`` attaches every outstanding semaphore
    wait to the single tail Drain instruction; the walrus build in this
    container rejects Drains with more than one sync wait ("Too many sync
    wait commands", CoreV3GenImpl setupSyncWait). Emit one single-wait NOP
    per outstanding proc on the SP queue ahead of the drain instead —
    program order on SP makes the bare drain equivalent.
    """

    def _drain_and_barrier(self, tick_clock, wait_clock):
        from concourse.vector_clock import ScopedClock, VectorClock

        gvec = list(tick_clock.global_clock)
        n = len(gvec)
        for p, tick in enumerate(gvec):
            if tick > 0:
                nop = self.nc.sync.nop(nofuse=True, hint=f"drainwait{p}")
                partial = [0] * n
                partial[p] = tick
                wait_clock.add_sem_waits(
                    nop.ins, ScopedClock({None: VectorClock(partial)})
                )
        self.nc.sync.drain()
        self.nc.all_engine_barrier()
        popped = self.nc._tile_sem_poison_stack.pop()
        assert popped is self._sem_poison
        self.nc.clear_and_free_semaphores(list(self.sems.allocated().values()))
        self.nc.all_engine_barrier()

def _legalize_waits(nc):
    """Split multi-wait instructions for this walrus build.

    The container's walrus rejects any instruction carrying more than one
    sync-wait command ("Too many sync wait commands"). Hoist all but the
    last wait of each instruction onto preceding NoOps on the same engine
    queue — queue program order makes this equivalent.
    """
    n = 0
    for f in nc.m.functions:
        for blk in f.blocks:
            insts = blk.instructions
            out = []
            changed = False
            for inst in insts:
                si = inst.sync_info
                if si is not None and len(si.on_wait) > 1:
                    waits = list(si.on_wait)
                    for w in waits[:-1]:
                        nop = mybir.InstNoOp(name=f"I-lw{n}")
                        n += 1
                        nop.engine = inst.engine
                        nop.sync_info = mybir.SyncInfo(on_wait=[w], on_update=[])
                        out.append(nop)
                    inst.sync_info = mybir.SyncInfo(
                        on_wait=[waits[-1]], on_update=list(si.on_update)
                    )
                    changed = True
                out.append(inst)
            if changed:
                blk.instructions = out


B, L, D, H = 4, 2048, 1024, 16
HD = D // H          # 64
NCORES = 8
HPG = 8              # heads per group (per core)
FG = HPG * HD        # 512 features per group
DT = D // 128        # 8 d-tiles
FT = FG // 128       # 4 feature tiles
LQ = L // 512        # 4 l_q chunks
LK = L // 128        # 16 l_k tiles


def _emit(ctx, tc):
    nc = tc.nc
    xq = nc.dram_tensor("xq", [D, L], BF16, kind="ExternalInput").ap()
    xk = nc.dram_tensor("xk", [D, L], BF16, kind="ExternalInput").ap()
    xv = nc.dram_tensor("xv", [D, L], BF16, kind="ExternalInput").ap()
    wq = nc.dram_tensor("wq", [D, FG], BF16, kind="ExternalInput").ap()
    wk = nc.dram_tensor("wk", [D, FG], BF16, kind="ExternalInput").ap()
    wv = nc.dram_tensor("wv", [D, FG], BF16, kind="ExternalInput").ap()
    wo = nc.dram_tensor("wo", [FG, D], BF16, kind="ExternalInput").ap()
    bqk = nc.dram_tensor("bqk", [2, FT, 128], F32, kind="ExternalInput").ap()
    bv = nc.dram_tensor("bv", [FG], F32, kind="ExternalInput").ap()
    out = nc.dram_tensor("out", [L, D], F32, kind="ExternalOutput").ap()

    singles = ctx.enter_context(tc.tile_pool(name="singles", bufs=1))
    wpool = ctx.enter_context(tc.tile_pool(name="wpool", bufs=9))
    xpool = ctx.enter_context(tc.tile_pool(name="xpool", bufs=10))
    qt_pool = ctx.enter_context(tc.tile_pool(name="qt_pool", bufs=FT))
    kt_pool = ctx.enter_context(tc.tile_pool(name="kt_pool", bufs=FT))
    v_pool = ctx.enter_context(tc.tile_pool(name="v_pool", bufs=LK))
    xt_pool = ctx.enter_context(tc.tile_pool(name="xt_pool", bufs=FT))
    pt_pool = ctx.enter_context(tc.tile_pool(name="pt_pool", bufs=15))
    rec_pool = ctx.enter_context(tc.tile_pool(name="rec_pool", bufs=3))
    out_pool = ctx.enter_context(tc.tile_pool(name="out_pool", bufs=3))
    acc_psum = ctx.enter_context(tc.tile_pool(name="acc_psum", bufs=4, space="PSUM"))
    e_psum = ctx.enter_context(tc.tile_pool(name="e_psum", bufs=2, space="PSUM"))

    # --- constants / weights resident in SBUF ---
    bias_qk = singles.tile([128, 2, FT], F32, name="bias_qk")
    for t in range(2):
        for ft in range(FT):
            src = bass.AP(
                tensor=bqk.tensor,
                offset=bqk.offset + (t * FT + ft) * 128,
                ap=[[1, 128], [1, 1]],
            )
            nc.sync.dma_start(out=bias_qk[:, t, ft : ft + 1], in_=src)

    bv_bc = singles.tile([128, FG], F32, name="bv_bc")
    bv_src = bass.AP(tensor=bv.tensor, offset=bv.offset, ap=[[0, 128], [1, FG]])
    nc.sync.dma_start(out=bv_bc, in_=bv_src)

    wo_sb = []
    for ft in range(FT):
        t_ = singles.tile([128, D], BF16, name=f"wo_sb{ft}")
        nc.sync.dma_start(out=t_, in_=wo[ft * 128 : (ft + 1) * 128, :])
        wo_sb.append(t_)

    # --- persistent activation tensors ---
    # V tiles hold [V_h (64 cols) | ones (64 cols)] per head, so a single
    # M=128 matmul per l_k tile accumulates both the attention output
    # (partitions 0:64) and the replicated softmax denominator (64:128).
    QT = [qt_pool.tile([128, L], BF16, name=f"qt{i}", tag="qt") for i in range(FT)]
    KT = [kt_pool.tile([128, L], BF16, name=f"kt{i}", tag="kt") for i in range(FT)]
    V = [v_pool.tile([128, HPG * 128], BF16, name=f"v{i}", tag="v") for i in range(LK)]
    XT = [xt_pool.tile([128, L], BF16, name=f"xt{i}", tag="xt") for i in range(FT)]
    for lt in range(LK):
        v3d = V[lt].rearrange("p (h c) -> p h c", c=128)
        nc.vector.memset(v3d[:, :, HD : 2 * HD], 1.0)

    # --- projections ---
    def load_w(wdram):
        w_sb = []
        for dt_ in range(DT):
            t_ = wpool.tile([128, FG], BF16, name="w_t", tag="w")
            nc.sync.dma_start(out=t_, in_=wdram[dt_ * 128 : (dt_ + 1) * 128, :])
            w_sb.append(t_)
        return w_sb

    def load_x_quarter(xdram, qrt):
        x_t = []
        for dt_ in range(DT):
            t_ = xpool.tile([128, 512], BF16, name="x_t", tag="x")
            nc.sync.dma_start(
                out=t_,
                in_=xdram[dt_ * 128 : (dt_ + 1) * 128, qrt * 512 : (qrt + 1) * 512],
            )
            x_t.append(t_)
        return x_t

    for ti, (xdram, wdram, out_sb) in enumerate(
        [(xq, wq, QT), (xk, wk, KT), (xv, wv, V)]
    ):
        w_sb = load_w(wdram)
        for qrt in range(LQ):
            x_t = load_x_quarter(xdram, qrt)
            if ti < 2:  # Q, K -> [feat, l] layout
                for ft in range(FT):
                    ps = acc_psum.tile([128, 512], F32, name="ps_proj", tag="acc")
                    for dt_ in range(DT):
                        nc.tensor.matmul(
                            ps,
                            lhsT=w_sb[dt_][:, ft * 128 : (ft + 1) * 128],
                            rhs=x_t[dt_],
                            start=(dt_ == 0),
                            stop=(dt_ == DT - 1),
                        )
                    nc.vector.tensor_scalar_add(
                        out_sb[ft][:, qrt * 512 : (qrt + 1) * 512],
                        ps,
                        bias_qk[:, ti, ft : ft + 1],
                    )
            else:  # V -> [l, feat] layout, bf16 + bias broadcast along partitions
                for lt4 in range(4):
                    ps = acc_psum.tile([128, FG], F32, name="ps_projv", tag="acc")
                    for dt_ in range(DT):
                        nc.tensor.matmul(
                            ps,
                            lhsT=x_t[dt_][:, lt4 * 128 : (lt4 + 1) * 128],
                            rhs=w_sb[dt_],
                            start=(dt_ == 0),
                            stop=(dt_ == DT - 1),
                        )
                    nc.vector.tensor_add(
                        V[qrt * 4 + lt4].rearrange("p (h c) -> p h c", c=128)[
                            :, :, 0:HD
                        ],
                        ps.rearrange("p (h c) -> p h c", c=HD),
                        bv_bc.rearrange("p (h c) -> p h c", c=HD),
                    )

    # --- attention + output projection, software-pipelined ---
    # Heads are processed in even/odd pairs (p -> heads 2p, 2p+1, same KT/QT
    # partition tile, partitions 0:64 and 64:128). A group is one l_k tile
    # of one (pair, l_q chunk) unit: the two heads' E.T matmuls hit
    # disjoint PE row groups, so they run concurrently and hide each
    # other's weight loads; they fill the two banks of a [128, 1024] psum
    # tile that a single ACT exp drains to bf16 P.T. Each head's X''
    # matmul ([V_h|ones].T @ P.T) accumulates into its own [128, 512] psum
    # tile (0:64 = X.T, 64:128 = replicated denominator). The group stream
    # runs through an 8-group software-pipeline ring (E of group j+8 is
    # emitted next to X'' of group j) so ACT, the softmax bottleneck,
    # never waits for PE. After the 8 heads of an l_q chunk finish, its
    # output-projection rows are emitted.
    units = [(p, lq) for lq in range(LQ) for p in range(HPG // 2)]
    NU = len(units)
    LOOKAHEAD = 12

    def emit_e_group(j):
        u, g = divmod(j, LK)
        p, lq = units[u]
        ep = e_psum.tile([128, 1024], F32, name="ep", tag="ep")
        for i in range(2):
            po = i * 64
            nc.tensor.matmul(
                ep[:, i * 512 : (i + 1) * 512],
                lhsT=KT[p][po : po + 64, g * 128 : (g + 1) * 128],
                rhs=QT[p][po : po + 64, lq * 512 : (lq + 1) * 512],
                tile_position=(po, 0),
                skip_group_check=True,
            )
        pt = pt_pool.tile([128, 2, 512], BF16, name="pt", tag="pt")
        nc.scalar.activation(
            out=pt,
            in_=ep.rearrange("p (a b) -> p a b", a=2),
            func=mybir.ActivationFunctionType.Exp,
            scale=0.125,
        )
        return pt

    def emit_x_group(j, xaccs, pt):
        u, g = divmod(j, LK)
        p, lq = units[u]
        for i in range(2):
            h = 2 * p + i
            nc.tensor.matmul(
                xaccs[i],
                lhsT=V[g][:, h * 128 : (h + 1) * 128],
                rhs=pt[:, i, :],
                start=(g == 0),
                stop=(g == LK - 1),
                skip_group_check=True,
            )

    def emit_tail(u, xaccs):
        p, lq = units[u]
        for i in range(2):
            po = i * 64
            rec = rec_pool.tile([128, 512], F32, name="rec", tag="rec")
            nc.vector.reciprocal(rec[64:128, :], xaccs[i][64:128, :])
            nc.vector.tensor_mul(
                XT[p][po : po + 64, lq * 512 : (lq + 1) * 512],
                xaccs[i][0:64, :],
                rec[64:128, :],
            )

    def emit_outproj_tile(lt, jt):
        ps = acc_psum.tile([128, 512], F32, name="ps_out", tag="acc")
        for ft_ in range(FT):
            nc.tensor.matmul(
                ps,
                lhsT=XT[ft_][:, lt * 128 : (lt + 1) * 128],
                rhs=wo_sb[ft_][:, jt * 512 : (jt + 1) * 512],
                start=(ft_ == 0),
                stop=(ft_ == FT - 1),
            )
        osb = out_pool.tile([128, 512], F32, name="osb", tag="osb")
        nc.vector.tensor_copy(osb, ps)
        nc.sync.dma_start(
            out=out[lt * 128 : (lt + 1) * 128, jt * 512 : (jt + 1) * 512],
            in_=osb,
        )

    NJ = NU * LK
    pts = {}
    xaccs = None
    pending_out = []
    for j in range(NJ + LOOKAHEAD):
        if j < NJ:
            pts[j] = emit_e_group(j)
        jx = j - LOOKAHEAD
        if 0 <= jx < NJ:
            u, g = divmod(jx, LK)
            if g == 0:
                xaccs = [
                    acc_psum.tile([128, 512], F32, name=f"xacc{i}", tag="acc")
                    for i in range(2)
                ]
            emit_x_group(jx, xaccs, pts.pop(jx))
            if g == LK - 1:
                emit_tail(u, xaccs)
                p, lq = units[u]
                if p == HPG // 2 - 1:
                    pending_out.extend(
                        (lt, jt)
                        for lt in range(lq * 4, (lq + 1) * 4)
                        for jt in range(2)
                    )
        # spread output-projection tiles one per ring step so their matmul
        # bursts never starve ACT of E-group work
        if pending_out and (j % 2 == 0 or j >= NJ):
            emit_outproj_tile(*pending_out.pop(0))
    while pending_out:
        emit_outproj_tile(*pending_out.pop(0))


def build_program():
    nc = bass.Bass("TRN2", target_bir_lowering=False, debug=False, num_devices=NCORES)
    with _TileContext(nc) as tc:
        with ExitStack() as ctx:
            _emit(ctx, tc)
    return nc


def make_in_maps(query, key, value, Wq, bq, Wk, bk, Wv, bv, Wo, bo):
    query = np.asarray(query, np.float32)
    key = np.asarray(key, np.float32)
    value = np.asarray(value, np.float32)
    xqs = [np.ascontiguousarray(query[b].T).astype(ml_dtypes.bfloat16) for b in range(B)]
    xks = [np.ascontiguousarray(key[b].T).astype(ml_dtypes.bfloat16) for b in range(B)]
    xvs = [np.ascontiguousarray(value[b].T).astype(ml_dtypes.bfloat16) for b in range(B)]
    in_maps = []
    for c in range(NCORES):
        b, g = divmod(c, 2)
        fs = slice(g * FG, (g + 1) * FG)
        in_maps.append(
            {
                "xq": xqs[b],
                "xk": xks[b],
                "xv": xvs[b],
                "wq": np.ascontiguousarray(np.asarray(Wq, np.float32)[fs, :].T).astype(ml_dtypes.bfloat16),
                "wk": np.ascontiguousarray(np.asarray(Wk, np.float32)[fs, :].T).astype(ml_dtypes.bfloat16),
                "wv": np.ascontiguousarray(np.asarray(Wv, np.float32)[fs, :].T).astype(ml_dtypes.bfloat16),
                "wo": np.ascontiguousarray(
                    np.asarray(Wo, np.float32)[:, fs].T
                ).astype(ml_dtypes.bfloat16),
                "bqk": np.stack(
                    [
                        np.asarray(bq, np.float32)[fs].reshape(FT, 128),
                        np.asarray(bk, np.float32)[fs].reshape(FT, 128),
                    ]
                ),
                "bv": np.ascontiguousarray(np.asarray(bv, np.float32)[fs]),
            }
        )
    return in_maps


def kernel(query, key, value, Wq, bq, Wk, bk, Wv, bv, Wo, bo, _trace=False):
    nc = build_program()
    _legalize_waits(nc)
    in_maps = make_in_maps(query, key, value, Wq, bq, Wk, bk, Wv, bv, Wo, bo)
    try:
        res = run_bass_kernel_spmd(
            nc, in_maps, core_ids=list(range(NCORES)), trace=_trace
        )
    except ModuleNotFoundError:
        res = run_bass_kernel_spmd(nc, in_maps, core_ids=list(range(NCORES)))
    full = np.empty((B, L, D), np.float32)
    bo32 = np.asarray(bo, np.float32)
    for b in range(B):
        full[b] = res.results[2 * b]["out"] + res.results[2 * b + 1]["out"] + bo32
    if _trace:
        kernel._last_trace = res
    return full


# revision 33
# speedup vs baseline: 1.0215x; 1.0215x over previous
"""Cross-attention (B=4, L=2048, D=1024, H=16) on 8 TRN2 NeuronCores.

Sharding: core c handles batch b = c//2 and head-group g = c%2 (8 heads,
512 projection features). Each core computes its heads' Q/K/V projections,
attention, and a partial output projection (contraction over its 512
features). Host sums the two partials per batch and adds the output bias.

Per-core layouts (host pre-arranged, matmul operands cast to bf16):
  xq/xk/xv [D=1024, L=2048]  activations transposed (contraction dim on
                             partitions for the projection matmuls), bf16
  wq/wk/wv [D=1024, F=512]   W[F,:].T  (d-major), bf16
  wo       [F=512, D=1024]   Wo[:,F].T (feat-major), bf16
  bqk      [2, 4, 128]       q/k biases reshaped for partition-dim loads
  bv       [512]             v bias (free-dim broadcast DMA)
Output: out [L=2048, D=1024] fp32 partial (x_g @ Wo[:,F].T), no bias.

On-device dataflow per core (all matmuls bf16 with fp32 psum accumulate):
  QT/KT [feat(4x128p), L] bf16 = (w-tile).T @ x-tile + bias
  V     [L(16x128p), 8*(V_h 64 | ones 64)] bf16 (ones interleaved so one
        M=128 matmul yields both attention output and softmax denominator)
  per (head, l_q chunk) unit, groups of 2 l_k tiles:
    E.T [l_k 128, 2*512] psum = KT_h_tile.T @ QT_h   (K=64)
    ACT exp(0.125 * E.T) reads both psum banks -> P.T bf16 in SBUF
        (no max subtraction: |E/8| < ~7 for these distributions)
    X'' [128, 512] psum += [V_h|ones].T @ P.T  (partitions 0:64 = X.T,
        64:128 = replicated denominator)
    DVE: rec = reciprocal(X''[64:128]); XT = X''[0:64] * rec
  The next unit's E groups are emitted interleaved with the current
  unit's X'' matmuls so ACT (the softmax bottleneck) never idles.
  out [l 128, j 512] psum = XT-tile.T @ wo, DVE copy, DMA -> DRAM
"""

from contextlib import ExitStack

import numpy as np
import ml_dtypes

import concourse.bass as bass
import concourse.tile as tile
import concourse.mybir as mybir
from concourse.bass_utils import run_bass_kernel_spmd

F32 = mybir.dt.float32
F32R = mybir.dt.float32r
BF16 = mybir.dt.bfloat16


class _TileContext(tile.TileContext):
    """TileContext whose kernel-tail drain splits its semaphore waits.

    The stock ``_drain_and_barrier`` attaches every outstanding semaphore
    wait to the single tail Drain instruction; the walrus build in this
    container rejects Drains with more than one sync wait ("Too many sync
    wait commands", CoreV3GenImpl setupSyncWait). Emit one single-wait NOP
    per outstanding proc on the SP queue ahead of the drain instead —
    program order on SP makes the bare drain equivalent.
    """

    def _drain_and_barrier(self, tick_clock, wait_clock):
        from concourse.vector_clock import ScopedClock, VectorClock

        gvec = list(tick_clock.global_clock)
        n = len(gvec)
        for p, tick in enumerate(gvec):
            if tick > 0:
                nop = self.nc.sync.nop(nofuse=True, hint=f"drainwait{p}")
                partial = [0] * n
                partial[p] = tick
                wait_clock.add_sem_waits(
                    nop.ins, ScopedClock({None: VectorClock(partial)})
                )
        self.nc.sync.drain()
        self.nc.all_engine_barrier()
        popped = self.nc._tile_sem_poison_stack.pop()
        assert popped is self._sem_poison
        self.nc.clear_and_free_semaphores(list(self.sems.allocated().values()))
        self.nc.all_engine_barrier()

def _legalize_waits(nc):
    """Split multi-wait instructions for this walrus build.

    The container's walrus rejects any instruction carrying more than one
    sync-wait command ("Too many sync wait commands"). Hoist all but the
    last wait of each instruction onto preceding NoOps on the same engine
    queue — queue program order makes this equivalent.
    """
    n = 0
    for f in nc.m.functions:
        for blk in f.blocks:
            insts = blk.instructions
            out = []
            changed = False
            for inst in insts:
                si = inst.sync_info
                if si is not None and len(si.on_wait) > 1:
                    waits = list(si.on_wait)
                    for w in waits[:-1]:
                        nop = mybir.InstNoOp(name=f"I-lw{n}")
                        n += 1
                        nop.engine = inst.engine
                        nop.sync_info = mybir.SyncInfo(on_wait=[w], on_update=[])
                        out.append(nop)
                    inst.sync_info = mybir.SyncInfo(
                        on_wait=[waits[-1]], on_update=list(si.on_update)
                    )
                    changed = True
                out.append(inst)
            if changed:
                blk.instructions = out


B, L, D, H = 4, 2048, 1024, 16
HD = D // H          # 64
NCORES = 8
HPG = 8              # heads per group (per core)
FG = HPG * HD        # 512 features per group
DT = D // 128        # 8 d-tiles
FT = FG // 128       # 4 feature tiles
LQ = L // 512        # 4 l_q chunks
LK = L // 128        # 16 l_k tiles


def _emit(ctx, tc):
    nc = tc.nc
    xq = nc.dram_tensor("xq", [D, L], BF16, kind="ExternalInput").ap()
    xk = nc.dram_tensor("xk", [D, L], BF16, kind="ExternalInput").ap()
    xv = nc.dram_tensor("xv", [D, L], BF16, kind="ExternalInput").ap()
    wq = nc.dram_tensor("wq", [D, FG], BF16, kind="ExternalInput").ap()
    wk = nc.dram_tensor("wk", [D, FG], BF16, kind="ExternalInput").ap()
    wv = nc.dram_tensor("wv", [D, FG], BF16, kind="ExternalInput").ap()
    wo = nc.dram_tensor("wo", [FG, D], BF16, kind="ExternalInput").ap()
    bqk = nc.dram_tensor("bqk", [2, FT, 128], F32, kind="ExternalInput").ap()
    bv = nc.dram_tensor("bv", [FG], F32, kind="ExternalInput").ap()
    out = nc.dram_tensor("out", [L, D], F32, kind="ExternalOutput").ap()

    singles = ctx.enter_context(tc.tile_pool(name="singles", bufs=1))
    wpool = ctx.enter_context(tc.tile_pool(name="wpool", bufs=9))
    xpool = ctx.enter_context(tc.tile_pool(name="xpool", bufs=10))
    qt_pool = ctx.enter_context(tc.tile_pool(name="qt_pool", bufs=FT))
    kt_pool = ctx.enter_context(tc.tile_pool(name="kt_pool", bufs=FT))
    v_pool = ctx.enter_context(tc.tile_pool(name="v_pool", bufs=LK))
    xt_pool = ctx.enter_context(tc.tile_pool(name="xt_pool", bufs=FT))
    pt_pool = ctx.enter_context(tc.tile_pool(name="pt_pool", bufs=11))
    rec_pool = ctx.enter_context(tc.tile_pool(name="rec_pool", bufs=3))
    out_pool = ctx.enter_context(tc.tile_pool(name="out_pool", bufs=3))
    acc_psum = ctx.enter_context(tc.tile_pool(name="acc_psum", bufs=4, space="PSUM"))
    e_psum = ctx.enter_context(tc.tile_pool(name="e_psum", bufs=2, space="PSUM"))

    # --- constants / weights resident in SBUF ---
    bias_qk = singles.tile([128, 2, FT], F32, name="bias_qk")
    for t in range(2):
        for ft in range(FT):
            src = bass.AP(
                tensor=bqk.tensor,
                offset=bqk.offset + (t * FT + ft) * 128,
                ap=[[1, 128], [1, 1]],
            )
            nc.sync.dma_start(out=bias_qk[:, t, ft : ft + 1], in_=src)

    bv_bc = singles.tile([128, FG], F32, name="bv_bc")
    bv_src = bass.AP(tensor=bv.tensor, offset=bv.offset, ap=[[0, 128], [1, FG]])
    nc.sync.dma_start(out=bv_bc, in_=bv_src)

    wo_sb = []
    for ft in range(FT):
        t_ = singles.tile([128, D], BF16, name=f"wo_sb{ft}")
        nc.sync.dma_start(out=t_, in_=wo[ft * 128 : (ft + 1) * 128, :])
        wo_sb.append(t_)

    # --- persistent activation tensors ---
    # V tiles hold [V_h (64 cols) | ones (64 cols)] per head, so a single
    # M=128 matmul per l_k tile accumulates both the attention output
    # (partitions 0:64) and the replicated softmax denominator (64:128).
    QT = [qt_pool.tile([128, L], BF16, name=f"qt{i}", tag="qt") for i in range(FT)]
    KT = [kt_pool.tile([128, L], BF16, name=f"kt{i}", tag="kt") for i in range(FT)]
    V = [v_pool.tile([128, HPG * 128], BF16, name=f"v{i}", tag="v") for i in range(LK)]
    XT = [xt_pool.tile([128, L], BF16, name=f"xt{i}", tag="xt") for i in range(FT)]
    for lt in range(LK):
        v3d = V[lt].rearrange("p (h c) -> p h c", c=128)
        nc.vector.memset(v3d[:, :, HD : 2 * HD], 1.0)

    # --- projections ---
    def load_w(wdram):
        w_sb = []
        for dt_ in range(DT):
            t_ = wpool.tile([128, FG], BF16, name="w_t", tag="w")
            nc.sync.dma_start(out=t_, in_=wdram[dt_ * 128 : (dt_ + 1) * 128, :])
            w_sb.append(t_)
        return w_sb

    def load_x_quarter(xdram, qrt):
        x_t = []
        for dt_ in range(DT):
            t_ = xpool.tile([128, 512], BF16, name="x_t", tag="x")
            nc.sync.dma_start(
                out=t_,
                in_=xdram[dt_ * 128 : (dt_ + 1) * 128, qrt * 512 : (qrt + 1) * 512],
            )
            x_t.append(t_)
        return x_t

    for ti, (xdram, wdram, out_sb) in enumerate(
        [(xq, wq, QT), (xk, wk, KT), (xv, wv, V)]
    ):
        w_sb = load_w(wdram)
        for qrt in range(LQ):
            x_t = load_x_quarter(xdram, qrt)
            if ti < 2:  # Q, K -> [feat, l] layout
                for ft in range(FT):
                    ps = acc_psum.tile([128, 512], F32, name="ps_proj", tag="acc")
                    for dt_ in range(DT):
                        nc.tensor.matmul(
                            ps,
                            lhsT=w_sb[dt_][:, ft * 128 : (ft + 1) * 128],
                            rhs=x_t[dt_],
                            start=(dt_ == 0),
                            stop=(dt_ == DT - 1),
                        )
                    nc.vector.tensor_scalar_add(
                        out_sb[ft][:, qrt * 512 : (qrt + 1) * 512],
                        ps,
                        bias_qk[:, ti, ft : ft + 1],
                    )
            else:  # V -> [l, feat] layout, bf16 + bias broadcast along partitions
                for lt4 in range(4):
                    ps = acc_psum.tile([128, FG], F32, name="ps_projv", tag="acc")
                    for dt_ in range(DT):
                        nc.tensor.matmul(
                            ps,
                            lhsT=x_t[dt_][:, lt4 * 128 : (lt4 + 1) * 128],
                            rhs=w_sb[dt_],
                            start=(dt_ == 0),
                            stop=(dt_ == DT - 1),
                        )
                    nc.vector.tensor_add(
                        V[qrt * 4 + lt4].rearrange("p (h c) -> p h c", c=128)[
                            :, :, 0:HD
                        ],
                        ps.rearrange("p (h c) -> p h c", c=HD),
                        bv_bc.rearrange("p (h c) -> p h c", c=HD),
                    )

    # --- attention + output projection, software-pipelined ---
    # Heads are processed in even/odd pairs (p -> heads 2p, 2p+1, same KT/QT
    # partition tile, partitions 0:64 and 64:128). A group is one l_k tile
    # of one (pair, l_q chunk) unit: the two heads' E.T matmuls hit
    # disjoint PE row groups, so they run concurrently and hide each
    # other's weight loads; they fill the two banks of a [128, 1024] psum
    # tile that a single ACT exp drains to bf16 P.T. Each head's X''
    # matmul ([V_h|ones].T @ P.T) accumulates into its own [128, 512] psum
    # tile (0:64 = X.T, 64:128 = replicated denominator). The group stream
    # runs through an 8-group software-pipeline ring (E of group j+8 is
    # emitted next to X'' of group j) so ACT, the softmax bottleneck,
    # never waits for PE. After the 8 heads of an l_q chunk finish, its
    # output-projection rows are emitted.
    units = [(p, lq) for lq in range(LQ) for p in range(HPG // 2)]
    NU = len(units)
    LOOKAHEAD = 8

    def emit_e_group(j):
        u, g = divmod(j, LK)
        p, lq = units[u]
        ep = e_psum.tile([128, 1024], F32, name="ep", tag="ep")
        for i in range(2):
            po = i * 64
            nc.tensor.matmul(
                ep[:, i * 512 : (i + 1) * 512],
                lhsT=KT[p][po : po + 64, g * 128 : (g + 1) * 128],
                rhs=QT[p][po : po + 64, lq * 512 : (lq + 1) * 512],
                tile_position=(po, 0),
                skip_group_check=True,
            )
        pt = pt_pool.tile([128, 2, 512], BF16, name="pt", tag="pt")
        nc.scalar.activation(
            out=pt,
            in_=ep.rearrange("p (a b) -> p a b", a=2),
            func=mybir.ActivationFunctionType.Exp,
            scale=0.125,
        )
        return pt

    def emit_x_group(j, xaccs, pt):
        u, g = divmod(j, LK)
        p, lq = units[u]
        for i in range(2):
            h = 2 * p + i
            nc.tensor.matmul(
                xaccs[i],
                lhsT=V[g][:, h * 128 : (h + 1) * 128],
                rhs=pt[:, i, :],
                start=(g == 0),
                stop=(g == LK - 1),
                skip_group_check=True,
            )

    def emit_tail(u, xaccs):
        p, lq = units[u]
        for i in range(2):
            po = i * 64
            rec = rec_pool.tile([128, 512], F32, name="rec", tag="rec")
            nc.vector.reciprocal(rec[64:128, :], xaccs[i][64:128, :])
            nc.vector.tensor_mul(
                XT[p][po : po + 64, lq * 512 : (lq + 1) * 512],
                xaccs[i][0:64, :],
                rec[64:128, :],
            )

    def emit_outproj_tile(lt, jt):
        ps = acc_psum.tile([128, 512], F32, name="ps_out", tag="acc")
        for ft_ in range(FT):
            nc.tensor.matmul(
                ps,
                lhsT=XT[ft_][:, lt * 128 : (lt + 1) * 128],
                rhs=wo_sb[ft_][:, jt * 512 : (jt + 1) * 512],
                start=(ft_ == 0),
                stop=(ft_ == FT - 1),
            )
        osb = out_pool.tile([128, 512], F32, name="osb", tag="osb")
        nc.vector.tensor_copy(osb, ps)
        nc.sync.dma_start(
            out=out[lt * 128 : (lt + 1) * 128, jt * 512 : (jt + 1) * 512],
            in_=osb,
        )

    NJ = NU * LK
    pts = {}
    xaccs = None
    pending_out = []
    for j in range(NJ + LOOKAHEAD):
        if j < NJ:
            pts[j] = emit_e_group(j)
        jx = j - LOOKAHEAD
        if 0 <= jx < NJ:
            u, g = divmod(jx, LK)
            if g == 0:
                xaccs = [
                    acc_psum.tile([128, 512], F32, name=f"xacc{i}", tag="acc")
                    for i in range(2)
                ]
            emit_x_group(jx, xaccs, pts.pop(jx))
            if g == LK - 1:
                emit_tail(u, xaccs)
                p, lq = units[u]
                if p == HPG // 2 - 1:
                    pending_out.extend(
                        (lt, jt)
                        for lt in range(lq * 4, (lq + 1) * 4)
                        for jt in range(2)
                    )
        # spread output-projection tiles one per ring step so their matmul
        # bursts never starve ACT of E-group work
        if pending_out and (j % 2 == 0 or j >= NJ):
            emit_outproj_tile(*pending_out.pop(0))
    while pending_out:
        emit_outproj_tile(*pending_out.pop(0))


def build_program():
    nc = bass.Bass("TRN2", target_bir_lowering=False, debug=False, num_devices=NCORES)
    with _TileContext(nc) as tc:
        with ExitStack() as ctx:
            _emit(ctx, tc)
    return nc


def make_in_maps(query, key, value, Wq, bq, Wk, bk, Wv, bv, Wo, bo):
    query = np.asarray(query, np.float32)
    key = np.asarray(key, np.float32)
    value = np.asarray(value, np.float32)
    xqs = [np.ascontiguousarray(query[b].T).astype(ml_dtypes.bfloat16) for b in range(B)]
    xks = [np.ascontiguousarray(key[b].T).astype(ml_dtypes.bfloat16) for b in range(B)]
    xvs = [np.ascontiguousarray(value[b].T).astype(ml_dtypes.bfloat16) for b in range(B)]
    in_maps = []
    for c in range(NCORES):
        b, g = divmod(c, 2)
        fs = slice(g * FG, (g + 1) * FG)
        in_maps.append(
            {
                "xq": xqs[b],
                "xk": xks[b],
                "xv": xvs[b],
                "wq": np.ascontiguousarray(np.asarray(Wq, np.float32)[fs, :].T).astype(ml_dtypes.bfloat16),
                "wk": np.ascontiguousarray(np.asarray(Wk, np.float32)[fs, :].T).astype(ml_dtypes.bfloat16),
                "wv": np.ascontiguousarray(np.asarray(Wv, np.float32)[fs, :].T).astype(ml_dtypes.bfloat16),
                "wo": np.ascontiguousarray(
                    np.asarray(Wo, np.float32)[:, fs].T
                ).astype(ml_dtypes.bfloat16),
                "bqk": np.stack(
                    [
                        np.asarray(bq, np.float32)[fs].reshape(FT, 128),
                        np.asarray(bk, np.float32)[fs].reshape(FT, 128),
                    ]
                ),
                "bv": np.ascontiguousarray(np.asarray(bv, np.float32)[fs]),
            }
        )
    return in_maps


def kernel(query, key, value, Wq, bq, Wk, bk, Wv, bv, Wo, bo, _trace=False):
    nc = build_program()
    _legalize_waits(nc)
    in_maps = make_in_maps(query, key, value, Wq, bq, Wk, bk, Wv, bv, Wo, bo)
    try:
        res = run_bass_kernel_spmd(
            nc, in_maps, core_ids=list(range(NCORES)), trace=_trace
        )
    except ModuleNotFoundError:
        res = run_bass_kernel_spmd(nc, in_maps, core_ids=list(range(NCORES)))
    full = np.empty((B, L, D), np.float32)
    bo32 = np.asarray(bo, np.float32)
    for b in range(B):
        full[b] = res.results[2 * b]["out"] + res.results[2 * b + 1]["out"] + bo32
    if _trace:
        kernel._last_trace = res
    return full
